# revision 45
# baseline (speedup 1.0000x reference)
"""EmergentSpinGlass fused kernel for 8 Trainium2 NeuronCores.

Reference computation (per batch b):
    s   = x @ W_spin.T + b_spin                       (N, D)
    mf  = mean_n s                                    (D,)
    g   = W_global @ mf                               (D,)   [same for all rows]
    EF  = s @ W_J.T                                   (N, D)
    A   = softmax(EF @ s.T / sqrt(D), axis=-1)        (N, N)
    LF  = A @ s                                       (N, D)
    out = tanh(beta * (s + g + LF))                   (N, D)

Sharding: 8 cores = 4 batches x 2 query-halves. Each core receives x^T for
its batch with its query half's rows permuted first (attention is
permutation-invariant over keys), computes s for all 2048 keys, and runs
the attention block for its 1024 queries. Weights are pre-transposed on
the host; all device matmuls contract over the SBUF partition dim.

Work split (v5): each core computes s only for its OWN 1024 keys (= its
queries) in bf16; the fp8 copy of that half is exchanged between the two
cores of a batch with a pairwise DRAM AllGather (plus an AllReduce for
the mean-field partials), overlapped with the EF matmuls that only need
local data. The gathered key order is rank-major on both cores — valid
because attention is permutation-invariant over keys as long as the
scores rhs and the local-field rhs use the same layout.

Precision plan:
  - s-path (x, W_spin, s^T, the final s-term SQ) in bf16/f32: the s term
    enters tanh directly, so it is kept at >=bf16 accuracy.
  - EF, scores and the local-field matmuls run in fp8e4m3 with
    perf_mode=DoubleRow (2 contraction k-tiles per instruction, ~1.4x PE
    throughput at 512-wide moving operands). W_J / W_global are pre-scaled
    by 256 on the host so their entries sit in e4m3's normal range; the
    extra 256 is divided out when copying EF/g from PSUM.
  - P (softmax weights ~1/2048) is scaled by 256 before the fp8 cast so it
    stays in e4m3 normal/denormal range; the local-field PSUM is therefore
    256*(g + LF). The s term is added as SQ = 256*s (f32) and tanh uses
    beta/256 as its scale. Measured end-to-end rel err ~3e-3, vs the 2e-2
    gate.

Structure, tuned from hardware profiles:
  - head: a ~100-matmul PE warmup on memset data flips the HAM clock gate
    (cold PE runs at 1.2GHz, warm at 2.4GHz) while the first inputs
    stream; x chunks go on the ACT HWDGE queue, weights on the sync
    queue, interleaved by kt-group so the first matmul pass waits on
    0.75MB, not 5MB. No gpsimd anywhere (identity is a host input, beta
    arrives pre-broadcast): gpsimd has ~6us of boot latency.
  - ST8own (fp8 copy of s^T for the attention matmuls) is produced by an
    ACT Identity pass reading the same phase-1 PSUM the DVE bias-add
    reads, so it costs no extra DVE time and no serial conversion pass.
  - one PSUM pool spans phase 1+2; fp8 PE transposes write with element
    step 2 (hardware requirement) grouped 4-to-a-bank before a single
    512-wide copy; phase 5 is software-pipelined: scores/exp of query
    tile i overlap the P-transpose + local-field matmuls of tile i-1; the
    last tile runs at 4x256 granularity to shorten the tail.
"""

import numpy as np
import ml_dtypes

import concourse.bass as bass
import concourse.tile as tile
from concourse import bacc, mybir
from concourse import bass_utils
from concourse.bass_interp import get_hw_module

F32 = mybir.dt.float32
BF16 = mybir.dt.bfloat16
F32R = mybir.dt.float32r
FP8 = mybir.dt.float8e4
ADD = mybir.AluOpType.add
MULT = mybir.AluOpType.mult
DR = mybir.MatmulPerfMode.DoubleRow
IDENT = mybir.ActivationFunctionType.Identity

B, N, D = 4, 2048, 1024
NQ = N // 2          # queries per core
KT = D // 128        # 8 contraction tiles
MT = N // 128        # 16 key tiles
QT = NQ // 128       # 8 query tiles
NCH = N // 512       # 4 key chunks of 512
SCALE = 1.0 / np.sqrt(np.float32(D))
WSC = 256.0          # host pre-scale on W_J / W_global (e4m3 range)
PSC = 256.0          # P scale before fp8 cast

LAST_RESULT = None   # BassKernelResults of the most recent run (for test.py)
_CACHED = {}


def _build(debug=False, hw=True):
    nc = bacc.Bacc(
        "TRN2",
        target_bir_lowering=False,
        debug=False,
        enable_asserts=False,
        num_devices=8,
    )
    xt_d = nc.dram_tensor("xt", [128, KT, NQ], BF16, kind="ExternalInput").ap()
    wspin_d = nc.dram_tensor("wspinT", [128, KT, D], BF16, kind="ExternalInput").ap()
    wj_d = nc.dram_tensor("wjT8", [128, KT, D], FP8, kind="ExternalInput").ap()
    wglob_d = nc.dram_tensor("wglobT8", [128, KT, D], FP8, kind="ExternalInput").ap()
    bspin_d = nc.dram_tensor("bspin", [128, KT], F32, kind="ExternalInput").ap()
    beta_d = nc.dram_tensor("beta", [128, 1], F32, kind="ExternalInput").ap()
    ident_d = nc.dram_tensor("ident", [128, 128], F32, kind="ExternalInput").ap()
    out_d = nc.dram_tensor("out", [NQ, D], F32, kind="ExternalOutput").ap()

    with tile.TileContext(nc) as tc:
        with (
            tc.tile_pool(name="const", bufs=1) as const,
            tc.tile_pool(name="longp", bufs=1) as longp,
            tc.tile_pool(name="stats", bufs=8) as stats,
        ):
            # warmup operand first: DVE memset runs right after the NEFF
            # starts. No gpsimd anywhere in this kernel: gpsimd has ~6us of
            # boot latency that otherwise gates the init barrier, so the
            # identity comes from DRAM and beta arrives pre-broadcast.
            warm8 = const.tile([128, 128], FP8)
            nc.vector.memset(warm8, 1.0)
            ident32 = const.tile([128, 128], F32)
            nc.sync.dma_start(out=ident32[:], in_=ident_d[:])
            ident_s = const.tile([128, 128], BF16)
            nc.vector.tensor_copy(ident_s[:], ident32[:])
            ident8 = const.tile([128, 128], FP8)
            nc.vector.tensor_copy(ident8[:], ident32[:])
            ones8 = const.tile([1, 128], FP8)
            nc.vector.memset(ones8, 1.0)
            beta_sb = const.tile([128, 1], F32)
            nc.sync.dma_start(out=beta_sb[:], in_=beta_d[:])
            # tanh scale: beta/PSC (the local-field psum carries a PSC factor)
            nc.vector.tensor_scalar_mul(beta_sb[:], beta_sb[:], 1.0 / PSC)
            bspin_sb = const.tile([128, KT], F32)
            nc.sync.dma_start(out=bspin_sb[:], in_=bspin_d[:])
            mf4 = const.tile([128, KT, NCH], F32)
            mf = const.tile([128, KT], F32)
            mfs8 = const.tile([128, KT], FP8)
            gT8 = const.tile([1, D], FP8)

            # s^T own key-half (= own queries) in bf16; each core computes
            # only its half, then core pairs exchange the fp8 copy via a
            # pairwise DRAM AllGather (the peer half is only ever consumed
            # in fp8: scores rhs / SN). ST8all holds the gathered keys in
            # RANK order (rank0 half then rank1 half) — attention is
            # permutation-invariant over keys, so both cores can use the
            # same layout; queries always read the rank-free ST8own/ST/SQ.
            ST = longp.tile([128, KT, NQ], BF16)  # s^T: [d-in-tile, d-tile, key]
            ST8own = longp.tile([128, KT, NQ], FP8)
            ST8all = longp.tile([128, KT, N], FP8)

            # pool stack: efp {EF8} > wjp {wj8} > ph1 {x chunks, wspin}
            efp_cm = tc.tile_pool(name="efp", bufs=1)
            efp = efp_cm.__enter__()
            wjp_cm = tc.tile_pool(name="wjp", bufs=1)
            wjp = wjp_cm.__enter__()

            # ---- Phase 1: s^T = W_spin^T(kxo) . x^T(kxn) + bias (own half
            # only, bf16); then core pairs AllGather the fp8 copy + partial
            # mean-field through DRAM while phase 2 runs on local data.
            # One PSUM pool spans all of phase 1 + 2 (no pool-drain barriers
            # between phases): "psA" = 4 accumulators for chunk 0's first
            # ot-wave kt-split, "ps" = rotating tiles for everything else.
            ps12_cm = tc.tile_pool(name="ps12", bufs=4, space="PSUM")
            ps1 = ps12_cm.__enter__()
            dram_cm = tc.tile_pool(name="dramx", bufs=1, space="DRAM")
            dram = dram_cm.__enter__()
            st8h = dram.tile([128, KT, NQ], FP8)
            st8g = dram.tile([2, 128, KT, NQ], FP8)
            mfh = dram.tile([128, KT], F32)
            mfg = dram.tile([128, KT], F32)

            with tc.tile_pool(name="ph1", bufs=1) as ph1:
                wspin_sb = ph1.tile([128, KT, D], BF16)
                xtc = {}

                def load_chunk(nch, k0=0, k1=KT):
                    if k0 == 0:
                        t = ph1.tile([128, KT, 512], BF16, name=f"xtc{nch}",
                                     tag="xtc", bufs=2)
                        xtc[nch] = t
                    t = xtc[nch]
                    # x streams on the ACT HWDGE queue; weights on sync
                    nc.scalar.dma_start(
                        out=t[:, k0:k1, :],
                        in_=xt_d[:, k0:k1, nch * 512:(nch + 1) * 512])

                # strict need-order on both queues, interleaved by kt-group
                # so the first matmul pass waits on 0.75MB, not 5MB.
                nc.sync.dma_start(out=wspin_sb[:, 0:2, :], in_=wspin_d[:, 0:2, :])
                load_chunk(0, 0, 2)
                nc.sync.dma_start(out=wspin_sb[:, 2:4, :], in_=wspin_d[:, 2:4, :])
                load_chunk(0, 2, 4)
                nc.sync.dma_start(out=wspin_sb[:, 4:8, :], in_=wspin_d[:, 4:8, :])
                load_chunk(0, 4, 8)
                load_chunk(1)
                wj8_sb = wjp.tile([128, KT, D], FP8)
                nc.sync.dma_start(out=wj8_sb[:], in_=wj_d[:])

                # chunk 0: ot-wave 0:4 runs kt-split passes on 4 dedicated
                # accumulators so the PE can start on the first 0.75MB of
                # DMA; a PE warmup on memset data precedes it so the HAM
                # clock gate is released (K=8/8) before real matmuls arrive.
                psA = [ps1.tile([128, 512], F32, name=f"psA{ot}",
                                tag=f"psA{ot}", bufs=1)
                       for ot in range(4)]
                # long enough to flip the HAM clock gate (~3.4us of
                # activity) and then keep the PE busy until the first x/W
                # DMA lands (DMA subsystem ramps for ~10us after launch)
                NWARM = 100
                for i in range(NWARM):
                    nc.tensor.matmul(
                        psA[0][:, 0:128], warm8[:], warm8[:],
                        start=(i == 0), stop=(i == NWARM - 1),
                    )
                for kt0, kt1 in ((0, 2), (2, 4), (4, 8)):
                    for ot in range(4):
                        for kt in range(kt0, kt1):
                            nc.tensor.matmul(
                                psA[ot][:],
                                wspin_sb[:, kt, ot * 128:(ot + 1) * 128],
                                xtc[0][:, kt, :],
                                start=(kt == 0), stop=(kt == KT - 1),
                            )

                def drain_own(ot, ps, nch):
                    sl = slice(nch * 512, (nch + 1) * 512)
                    # bias add (DVE) + rowsum chunk; ACT makes the fp8 copy
                    nc.vector.tensor_scalar(
                        out=ST[:, ot, sl],
                        in0=ps[:],
                        scalar1=bspin_sb[:, ot:ot + 1],
                        scalar2=None,
                        op0=ADD, op1=ADD,
                        accum_out=mf4[:, ot, nch:nch + 1],
                    )
                    nc.scalar.activation(
                        out=ST8own[:, ot, sl], in_=ps[:],
                        func=IDENT, bias=bspin_sb[:, ot:ot + 1], scale=1.0,
                    )

                for ot in range(4):
                    drain_own(ot, psA[ot], 0)
                for ot in range(4, KT):
                    ps = ps1.tile([128, 512], F32)
                    for kt in range(KT):
                        nc.tensor.matmul(
                            ps[:],
                            wspin_sb[:, kt, ot * 128:(ot + 1) * 128],
                            xtc[0][:, kt, :],
                            start=(kt == 0), stop=(kt == KT - 1),
                        )
                    drain_own(ot, ps, 0)
                # ship chunk 0's fp8 half to DRAM while chunk 1 computes
                nc.sync.dma_start(out=st8h[:, :, 0:512], in_=ST8own[:, :, 0:512])
                for ot in range(KT):
                    ps = ps1.tile([128, 512], F32)
                    for kt in range(KT):
                        nc.tensor.matmul(
                            ps[:],
                            wspin_sb[:, kt, ot * 128:(ot + 1) * 128],
                            xtc[1][:, kt, :],
                            start=(kt == 0), stop=(kt == KT - 1),
                        )
                    drain_own(ot, ps, 1)
                nc.sync.dma_start(out=st8h[:, :, 512:1024],
                                  in_=ST8own[:, :, 512:1024])
                for ot in range(KT):
                    nc.vector.reduce_sum(
                        out=mf[:, ot:ot + 1], in_=mf4[:, ot, 0:2],
                        axis=mybir.AxisListType.X,
                    )
                nc.sync.dma_start(out=mfh[:], in_=mf[:])

                # pairwise exchange: ST8 halves (bypass gather) + mf (add)
                grps = [[0, 1], [2, 3], [4, 5], [6, 7]]
                nc.gpsimd.collective_compute(
                    "AllGather", mybir.AluOpType.bypass, replica_groups=grps,
                    ins=[st8h.opt()], outs=[st8g.opt()],
                )
                nc.gpsimd.collective_compute(
                    "AllReduce", ADD, replica_groups=grps,
                    ins=[mfh.opt()], outs=[mfg.opt()],
                )
                nc.scalar.dma_start(out=ST8all[:, :, 0:NQ], in_=st8g[0])
                nc.scalar.dma_start(out=ST8all[:, :, NQ:N], in_=st8g[1])
                mft = stats.tile([128, KT], F32)
                nc.sync.dma_start(out=mft[:], in_=mfg[:])
                # mfs8 = 16*mf (e4m3 normal range); g psum then carries
                # WSC*16 = 4096x, copied out with 1/16 -> gT8 = 256*g
                nc.vector.tensor_scalar_mul(mfs8[:], mft[:], 16.0 / N)

            dram_cm.__exit__(None, None, None)

            # ---- Phase 2: EF8 = (W_J8^T . ST8own)/WSC  (DoubleRow fp8) ----
            EF8 = efp.tile([128, KT, NQ], FP8)  # [d-in-tile, d-tile, query]
            with tc.tile_pool(name="ph2", bufs=1) as ph2:
                wglob8_sb = ph2.tile([128, KT, D], FP8)
                nc.sync.dma_start(out=wglob8_sb[:], in_=wglob_d[:])

                for ot in range(KT):
                    for ch in range(2):
                        ps = ps1.tile([128, 512], F32)
                        for dt_ in range(0, KT, 2):
                            nc.tensor.matmul(
                                ps[:],
                                wj8_sb[:, dt_:dt_ + 2, ot * 128:(ot + 1) * 128],
                                ST8own[:, dt_:dt_ + 2,
                                       ch * 512:(ch + 1) * 512],
                                start=(dt_ == 0), stop=(dt_ == KT - 2),
                                perf_mode=DR,
                            )
                        nc.vector.tensor_scalar_mul(
                            EF8[:, ot, ch * 512:(ch + 1) * 512], ps[:], 1.0 / WSC
                        )

                # g^T = mf^T . W_global^T (fp8, tiny; rides the rotating tag)
                for ch in range(2):
                    gp = ps1.tile([128, 512], F32, name="ps", tag="ps")
                    for dt_ in range(KT):
                        nc.tensor.matmul(
                            gp[0:1, :],
                            mfs8[:, dt_:dt_ + 1],
                            wglob8_sb[:, dt_, ch * 512:(ch + 1) * 512],
                            start=(dt_ == 0), stop=(dt_ == KT - 1),
                        )
                    nc.vector.tensor_scalar_mul(
                        gT8[:, ch * 512:(ch + 1) * 512], gp[0:1, :], 1.0 / 16.0
                    )
            ps12_cm.__exit__(None, None, None)
            wjp_cm.__exit__(None, None, None)

            # ---- Phase 3+4+5: scores (fp8 DR) -> softmax -> transposes ->
            # pipelined P-transpose + local-field (fp8 DR) ----
            with tc.tile_pool(name="att_sn", bufs=1) as att_sn:
                SN8 = att_sn.tile([128, MT, D], FP8)   # [key-in-tile, key-tile, d]
                SQ = att_sn.tile([128, QT, D], F32)    # 256*s for own queries
                with (
                    tc.tile_pool(name="work", bufs=2) as work,
                    tc.tile_pool(name="ps5s", bufs=1, space="PSUM") as ps5s,
                ):
                    def scores_softmax(qt):
                        q0 = qt * 128
                        ps_s = ps5s.tile([128, NCH, 512], F32)
                        P_sb = work.tile([128, N], FP8, bufs=3)
                        rs4 = stats.tile([128, NCH], F32)
                        for mch in range(NCH):
                            for dt_ in range(0, KT, 2):
                                nc.tensor.matmul(
                                    ps_s[:, mch, :],
                                    EF8[:, dt_:dt_ + 2, q0:q0 + 128],
                                    ST8all[:, dt_:dt_ + 2,
                                           mch * 512:(mch + 1) * 512],
                                    start=(dt_ == 0), stop=(dt_ == KT - 2),
                                    perf_mode=DR,
                                )
                            # no max subtraction: |scores|*SCALE < ~2 here
                            nc.scalar.activation(
                                out=P_sb[:, mch * 512:(mch + 1) * 512],
                                in_=ps_s[:, mch, :],
                                func=mybir.ActivationFunctionType.Exp,
                                bias=0.0, scale=float(SCALE),
                                accum_out=rs4[:, mch:mch + 1],
                            )
                        rs = stats.tile([128, 1], F32)
                        nc.vector.reduce_sum(out=rs[:], in_=rs4[:],
                                             axis=mybir.AxisListType.X)
                        rinv = stats.tile([128, 1], F32)
                        nc.vector.reciprocal(rinv[:], rs[:])
                        nc.vector.tensor_scalar_mul(rinv[:], rinv[:], PSC)
                        nc.vector.tensor_scalar_mul(P_sb[:], P_sb[:], rinv[:])
                        return P_sb

                    live = {}
                    live[0] = scores_softmax(0)
                    live[1] = scores_softmax(1)

                    # transposes: own-query tiles go via f32r (SQ needs f32
                    # precision); the rest transpose ST8 directly in fp8.
                    with tc.tile_pool(name="ps3", bufs=2, space="PSUM") as ps3:
                        # SN8 comes from the gathered (rank-ordered) fp8 s^T;
                        # SQ needs the core's own rows at bf16 precision, so
                        # query tiles get an extra bf16 transpose of ST.
                        for mt in range(MT):
                            for dq in range(KT // 4):
                                dsl4 = slice(dq * 512, (dq + 1) * 512)
                                # fp8 PE transpose needs element step 2
                                # in the PSUM output AP (walrus verifier)
                                tp8 = ps3.tile([128, 4, 128, 2], FP8)
                                for j in range(4):
                                    nc.tensor.transpose(
                                        tp8[:, j, :, 0],
                                        ST8all[:, dq * 4 + j,
                                               mt * 128:(mt + 1) * 128],
                                        ident8[:],
                                    )
                                if (mt + dq) % 2 == 0:
                                    nc.scalar.copy(
                                        SN8[:, mt, dsl4], tp8[:, :, :, 0])
                                else:
                                    nc.vector.tensor_copy(
                                        SN8[:, mt, dsl4], tp8[:, :, :, 0])
                                if mt < QT:
                                    tp = ps3.tile([128, 4, 128], BF16)
                                    for j in range(4):
                                        nc.tensor.transpose(
                                            tp[:, j, :],
                                            ST[:, dq * 4 + j,
                                               mt * 128:(mt + 1) * 128],
                                            ident_s[:],
                                        )
                                    nc.vector.tensor_scalar_mul(
                                        SQ[:, mt, dsl4], tp[:], PSC)

                    ps5t_cm = tc.tile_pool(name="ps5t", bufs=2, space="PSUM")
                    ps5t = ps5t_cm.__enter__()
                    ps5l_cm = tc.tile_pool(name="ps5l", bufs=2, space="PSUM")
                    ps5l = ps5l_cm.__enter__()

                    def pt_lf(qt, P_sb, fine=False):
                        q0 = qt * 128
                        PT = work.tile([128, MT, 128], FP8)
                        for mq in range(MT // 4):
                            tp2 = ps5t.tile([128, 4, 128, 2], FP8)
                            for j in range(4):
                                mt = mq * 4 + j
                                nc.tensor.transpose(
                                    tp2[:, j, :, 0],
                                    P_sb[:, mt * 128:(mt + 1) * 128],
                                    ident8[:],
                                )
                            nc.vector.tensor_copy(
                                PT[:, mq * 4:(mq + 1) * 4, :], tp2[:, :, :, 0])
                        # the very last tile runs at 4x256 so its final
                        # tanh+store chain (the kernel tail) is shorter
                        nd = 4 if fine else 2
                        w = D // nd
                        for dch in range(nd):
                            dsl = slice(dch * w, (dch + 1) * w)
                            plf = ps5l.tile([128, 512], F32)
                            # g term (broadcast over rows via rank-1 matmul)
                            nc.tensor.matmul(
                                plf[:, 0:w], ones8[:], gT8[:, dsl],
                                start=True, stop=False,
                            )
                            for mt in range(0, MT, 2):
                                nc.tensor.matmul(
                                    plf[:, 0:w], PT[:, mt:mt + 2, :],
                                    SN8[:, mt:mt + 2, dsl],
                                    start=False, stop=(mt == MT - 2),
                                    perf_mode=DR,
                                )
                            # psum = 256*(g+LF); add 256*s, tanh(beta/256 * .)
                            z = work.tile([128, 512], F32)
                            nc.vector.tensor_add(
                                z[:, 0:w], plf[:, 0:w], SQ[:, qt, dsl])
                            osb = work.tile([128, 512], F32, name="osb",
                                            tag="osb", bufs=4)
                            nc.scalar.activation(
                                out=osb[:, 0:w], in_=z[:, 0:w],
                                func=mybir.ActivationFunctionType.Tanh,
                                bias=0.0, scale=beta_sb[:],
                            )
                            nc.sync.dma_start(
                                out=out_d[q0:q0 + 128, dsl], in_=osb[:, 0:w])

                    for i in range(2, QT + 2):
                        if i < QT:
                            live[i] = scores_softmax(i)
                        pt_lf(i - 2, live.pop(i - 2), fine=(i == QT + 1))
                    ps5l_cm.__exit__(None, None, None)
                    ps5t_cm.__exit__(None, None, None)

            efp_cm.__exit__(None, None, None)

    nc.compile()
    if hw:
        nc.m = get_hw_module(nc.m)
    return nc


def _tile_kxm(a, np_dt):
    """(K, M) row-major -> [128, K//128, M] with k = kt*128 + p."""
    k, m = a.shape
    return np.ascontiguousarray(
        a.reshape(k // 128, 128, m).transpose(1, 0, 2)
    ).astype(np_dt)


def kernel(x, W_spin, b_spin, W_global, W_J, beta):
    global LAST_RESULT
    x = np.asarray(x, dtype=np.float32)
    W_spin = np.asarray(W_spin, dtype=np.float32)
    b_spin = np.asarray(b_spin, dtype=np.float32)
    W_global = np.asarray(W_global, dtype=np.float32)
    W_J = np.asarray(W_J, dtype=np.float32)
    beta = np.asarray(beta, dtype=np.float32)

    if "fp8" not in _CACHED:
        _CACHED["fp8"] = _build()
    nc = _CACHED["fp8"]

    wspinT = _tile_kxm(W_spin.T, ml_dtypes.bfloat16)   # W_spin.T is (k, o)
    wjT8 = _tile_kxm(WSC * W_J.T, ml_dtypes.float8_e4m3)
    wglobT8 = _tile_kxm(WSC * W_global.T, ml_dtypes.float8_e4m3)
    bspin = np.ascontiguousarray(b_spin.reshape(KT, 128).T).astype(np.float32)
    beta_h = np.broadcast_to(beta.reshape(1, 1), (128, 1)).astype(np.float32)
    beta_h = np.ascontiguousarray(beta_h)
    ident_h = np.eye(128, dtype=np.float32)

    # per-half x^T tiles: core (b,h) computes s in f32r for its own half
    # and in fp8 for the peer half
    xt_half = [_tile_kxm(np.ascontiguousarray(x[b, h * NQ:(h + 1) * NQ].T),
                         np.float32).astype(ml_dtypes.bfloat16)
               for b in range(B) for h in range(2)]

    in_maps = []
    for core in range(8):
        b, h = divmod(core, 2)
        in_maps.append({
            "xt": xt_half[2 * b + h],
            "wspinT": wspinT,
            "wjT8": wjT8, "wglobT8": wglobT8,
            "bspin": bspin, "beta": beta_h, "ident": ident_h,
        })

    LAST_RESULT = bass_utils.run_bass_kernel_spmd(
        nc, in_maps, core_ids=list(range(8))
    )

    out = np.empty((B, N, D), dtype=np.float32)
    for core in range(8):
        b, h = divmod(core, 2)
        out[b, h * NQ:(h + 1) * NQ, :] = LAST_RESULT.results[core]["out"]
    return out


# revision 46
# speedup vs baseline: 1.2167x; 1.2167x over previous
"""EmergentSpinGlass fused kernel for 8 Trainium2 NeuronCores.

Reference computation (per batch b):
    s   = x @ W_spin.T + b_spin                       (N, D)
    mf  = mean_n s                                    (D,)
    g   = W_global @ mf                               (D,)   [same for all rows]
    EF  = s @ W_J.T                                   (N, D)
    A   = softmax(EF @ s.T / sqrt(D), axis=-1)        (N, N)
    LF  = A @ s                                       (N, D)
    out = tanh(beta * (s + g + LF))                   (N, D)

Sharding: 8 cores = 4 batches x 2 query-halves. Each core receives x^T for
its batch with its query half's rows permuted first (attention is
permutation-invariant over keys), computes s for all 2048 keys, and runs
the attention block for its 1024 queries. Weights are pre-transposed on
the host; all device matmuls contract over the SBUF partition dim.

Precision plan (v2, fp8 attention):
  - s-path (x, W_spin, s^T) stays float32r (tf32-like): the s term enters
    tanh directly, so it needs ~1e-3 accuracy.
  - EF, scores and the local-field matmuls run in fp8e4m3 with
    perf_mode=DoubleRow (2 contraction k-tiles per instruction, ~1.4x PE
    throughput at 512-wide moving operands). W_J / W_global are pre-scaled
    by 256 on the host so their entries sit in e4m3's normal range; the
    extra 256 is divided out when copying EF/g from PSUM.
  - P (softmax weights ~1/2048) is scaled by 256 before the fp8 cast so it
    stays in e4m3 normal/denormal range; the local-field PSUM is therefore
    256*(g + LF). The s term is added as SQ = 256*s (f32) and tanh uses
    beta/256 as its scale. Predicted end-to-end rel err ~1.2e-3 (numpy
    simulation of the quantization pipeline), vs the 2e-2 gate.

Structure, tuned from hardware profiles:
  - head: the first matmul pass needs only W_spin[kt0:2] + x chunk0[kt0:2]
    (1.5MB); x chunks stream on the ACT HWDGE queue while weights use the
    sync queue, and chunk 0 is loaded in kt-groups so phase 1 starts as
    soon as the first 1.5MB lands (bare-queue baseline waited 20us).
  - ST8 (fp8 copy of s^T for the attention matmuls) is produced by an ACT
    Identity pass reading the same phase-1 PSUM the DVE bias-add reads, so
    it costs no extra DVE time and no serial conversion pass.
  - phase 5 is software-pipelined: scores/exp of query tile i overlap the
    P-transpose + local-field matmuls of tile i-1.
  - PE transposes write 4 tiles into one PSUM bank before a single
    512-wide copy (copy cost is latency-dominated).
"""

import numpy as np
import ml_dtypes

import concourse.bass as bass
import concourse.tile as tile
from concourse import bacc, mybir
from concourse import bass_utils
from concourse.bass_interp import get_hw_module

F32 = mybir.dt.float32
BF16 = mybir.dt.bfloat16
F32R = mybir.dt.float32r
FP8 = mybir.dt.float8e4
ADD = mybir.AluOpType.add
MULT = mybir.AluOpType.mult
DR = mybir.MatmulPerfMode.DoubleRow
IDENT = mybir.ActivationFunctionType.Identity

B, N, D = 4, 2048, 1024
NQ = N // 2          # queries per core
KT = D // 128        # 8 contraction tiles
MT = N // 128        # 16 key tiles
QT = NQ // 128       # 8 query tiles
NCH = N // 512       # 4 key chunks of 512
SCALE = 1.0 / np.sqrt(np.float32(D))
WSC = 256.0          # host pre-scale on W_J / W_global (e4m3 range)
PSC = 256.0          # P scale before fp8 cast

LAST_RESULT = None   # BassKernelResults of the most recent run (for test.py)
_CACHED = {}


def _build(debug=False, hw=True):
    nc = bacc.Bacc(
        "TRN2",
        target_bir_lowering=False,
        debug=False,
        enable_asserts=False,
        num_devices=8,
    )
    xt_d = nc.dram_tensor("xt", [128, KT, NQ], BF16, kind="ExternalInput").ap()
    xt8_d = nc.dram_tensor("xt8", [128, KT, NQ], FP8, kind="ExternalInput").ap()
    wspin_d = nc.dram_tensor("wspinT", [128, KT, D], BF16, kind="ExternalInput").ap()
    wspin8_d = nc.dram_tensor("wspinT8", [128, KT, D], FP8, kind="ExternalInput").ap()
    wj_d = nc.dram_tensor("wjT8", [128, KT, D], FP8, kind="ExternalInput").ap()
    wglob_d = nc.dram_tensor("wglobT8", [128, KT, D], FP8, kind="ExternalInput").ap()
    bspin_d = nc.dram_tensor("bspin", [128, KT], F32, kind="ExternalInput").ap()
    beta_d = nc.dram_tensor("beta", [128, 1], F32, kind="ExternalInput").ap()
    ident_d = nc.dram_tensor("ident", [128, 128], F32, kind="ExternalInput").ap()
    out_d = nc.dram_tensor("out", [NQ, D], F32, kind="ExternalOutput").ap()

    with tile.TileContext(nc) as tc:
        with (
            tc.tile_pool(name="const", bufs=1) as const,
            tc.tile_pool(name="longp", bufs=1) as longp,
            tc.tile_pool(name="stats", bufs=8) as stats,
        ):
            # warmup operand first: DVE memset runs right after the NEFF
            # starts. No gpsimd anywhere in this kernel: gpsimd has ~6us of
            # boot latency that otherwise gates the init barrier, so the
            # identity comes from DRAM and beta arrives pre-broadcast.
            warm8 = const.tile([128, 128], FP8)
            nc.vector.memset(warm8, 1.0)
            ident32 = const.tile([128, 128], F32)
            nc.sync.dma_start(out=ident32[:], in_=ident_d[:])
            ident_s = const.tile([128, 128], BF16)
            nc.vector.tensor_copy(ident_s[:], ident32[:])
            ident8 = const.tile([128, 128], FP8)
            nc.vector.tensor_copy(ident8[:], ident32[:])
            ones8 = const.tile([1, 128], FP8)
            nc.vector.memset(ones8, 1.0)
            beta_sb = const.tile([128, 1], F32)
            nc.sync.dma_start(out=beta_sb[:], in_=beta_d[:])
            # tanh scale: beta/PSC (the local-field psum carries a PSC factor)
            nc.vector.tensor_scalar_mul(beta_sb[:], beta_sb[:], 1.0 / PSC)
            bspin_sb = const.tile([128, KT], F32)
            nc.sync.dma_start(out=bspin_sb[:], in_=bspin_d[:])
            mf4 = const.tile([128, KT, NCH], F32)
            mf = const.tile([128, KT], F32)
            mfs8 = const.tile([128, KT], FP8)
            gT8 = const.tile([1, D], FP8)

            # s^T own key-half (= own queries) in f32r; full s^T in fp8.
            # keys 0:NQ are the core's own rows, NQ:N the peer half — the
            # peer half is only ever consumed in fp8 (scores rhs / SN), so
            # it is computed directly in fp8 from host-quantized x8/W8.
            ST = longp.tile([128, KT, NQ], BF16)  # s^T: [d-in-tile, d-tile, key]
            ST8 = longp.tile([128, KT, N], FP8)   # fp8 s^T, all keys

            # pool stack: efp {EF8} > wjp {wj8} > ph1 {x chunks, wspin}
            efp_cm = tc.tile_pool(name="efp", bufs=1)
            efp = efp_cm.__enter__()
            wjp_cm = tc.tile_pool(name="wjp", bufs=1)
            wjp = wjp_cm.__enter__()

            # ---- Phase 1: s^T = W_spin^T(kxo) . x^T(kxn) + bias; mf; ST8.
            # The fp8 peer half runs FIRST: its critical DMA is 0.375MB
            # (wspin8[0:2] + x8[0:2]) so the PE starts right after launch
            # while the 6MB bf16 stream for the own half lands underneath.
            # One PSUM pool spans all of phase 1 + 2 (no pool-drain barriers
            # between phases): "psA" = 4 accumulators for chunk A's first
            # ot-wave kt-split, "ps" = rotating tiles for everything else.
            ps12_cm = tc.tile_pool(name="ps12", bufs=4, space="PSUM")
            ps1 = ps12_cm.__enter__()

            with tc.tile_pool(name="ph1", bufs=1) as ph1:
                wspin_sb = ph1.tile([128, KT, D], BF16)
                wspin8_sb = ph1.tile([128, KT, D], FP8)
                xt8c = ph1.tile([128, KT, NQ], FP8)
                xtc = {}

                def load_chunk(nch, k0=0, k1=KT):
                    if k0 == 0:
                        t = ph1.tile([128, KT, 512], BF16, name=f"xtc{nch}",
                                     tag="xtc", bufs=2)
                        xtc[nch] = t
                    t = xtc[nch]
                    # x streams on the ACT HWDGE queue; weights on sync
                    nc.scalar.dma_start(
                        out=t[:, k0:k1, :],
                        in_=xt_d[:, k0:k1, nch * 512:(nch + 1) * 512])

                # strict need-order on both queues: fp8 weights/x first
                # (tiny), then the f32r stream interleaved by kt-group.
                nc.sync.dma_start(out=wspin8_sb[:, 0:2, :], in_=wspin8_d[:, 0:2, :])
                nc.scalar.dma_start(out=xt8c[:, 0:2, :], in_=xt8_d[:, 0:2, :])
                nc.sync.dma_start(out=wspin8_sb[:, 2:8, :], in_=wspin8_d[:, 2:8, :])
                nc.scalar.dma_start(out=xt8c[:, 2:8, :], in_=xt8_d[:, 2:8, :])
                nc.sync.dma_start(out=wspin_sb[:, 0:2, :], in_=wspin_d[:, 0:2, :])
                load_chunk(0, 0, 2)
                nc.sync.dma_start(out=wspin_sb[:, 2:4, :], in_=wspin_d[:, 2:4, :])
                load_chunk(0, 2, 4)
                nc.sync.dma_start(out=wspin_sb[:, 4:8, :], in_=wspin_d[:, 4:8, :])
                load_chunk(0, 4, 8)
                load_chunk(1)
                wj8_sb = wjp.tile([128, KT, D], FP8)
                nc.sync.dma_start(out=wj8_sb[:], in_=wj_d[:])

                # fp8 peer-half chunk A (keys NQ:NQ+512): ot-wave 0:4 runs
                # kt-split passes on 4 dedicated accumulators so the PE can
                # start on the first 0.375MB of DMA; a PE warmup on memset
                # data precedes it so the HAM clock gate is released
                # (K=8/8) before the real matmuls arrive.
                psA = [ps1.tile([128, 512], F32, name=f"psA{ot}",
                                tag=f"psA{ot}", bufs=1)
                       for ot in range(4)]
                # long enough to flip the HAM clock gate (~3.4us of
                # activity) and keep the PE busy until the first fp8 x/W
                # DMA lands (~17us; the DMA subsystem ramps for ~10us)
                NWARM = 100
                for i in range(NWARM):
                    nc.tensor.matmul(
                        psA[0][:, 0:128], warm8[:], warm8[:],
                        start=(i == 0), stop=(i == NWARM - 1),
                    )
                for kt0, kt1 in ((0, 2), (2, 4), (4, 8)):
                    for ot in range(4):
                        for kt in range(kt0, kt1, 2):
                            nc.tensor.matmul(
                                psA[ot][:],
                                wspin8_sb[:, kt:kt + 2,
                                          ot * 128:(ot + 1) * 128],
                                xt8c[:, kt:kt + 2, 0:512],
                                start=(kt == 0), stop=(kt == KT - 2),
                                perf_mode=DR,
                            )
                for ot in range(4):
                    # ACT: ST8 = psum/WSC + bias, rowsum into mf4
                    nc.scalar.activation(
                        out=ST8[:, ot, NQ:NQ + 512], in_=psA[ot][:],
                        func=IDENT, bias=bspin_sb[:, ot:ot + 1],
                        scale=1.0 / WSC,
                        accum_out=mf4[:, ot, 2:3],
                    )

                if True:
                    # chunk A ot-wave 4:8, then fp8 peer-half chunk B
                    for ot in range(4, KT):
                        ps = ps1.tile([128, 512], F32)
                        for kt in range(0, KT, 2):
                            nc.tensor.matmul(
                                ps[:],
                                wspin8_sb[:, kt:kt + 2, ot * 128:(ot + 1) * 128],
                                xt8c[:, kt:kt + 2, 0:512],
                                start=(kt == 0), stop=(kt == KT - 2),
                                perf_mode=DR,
                            )
                        nc.scalar.activation(
                            out=ST8[:, ot, NQ:NQ + 512], in_=ps[:],
                            func=IDENT, bias=bspin_sb[:, ot:ot + 1],
                            scale=1.0 / WSC,
                            accum_out=mf4[:, ot, 2:3],
                        )
                    for ot in range(KT):
                        ps = ps1.tile([128, 512], F32)
                        for kt in range(0, KT, 2):
                            nc.tensor.matmul(
                                ps[:],
                                wspin8_sb[:, kt:kt + 2, ot * 128:(ot + 1) * 128],
                                xt8c[:, kt:kt + 2, 512:1024],
                                start=(kt == 0), stop=(kt == KT - 2),
                                perf_mode=DR,
                            )
                        nc.scalar.activation(
                            out=ST8[:, ot, NQ + 512:N], in_=ps[:],
                            func=IDENT, bias=bspin_sb[:, ot:ot + 1],
                            scale=1.0 / WSC,
                            accum_out=mf4[:, ot, 3:4],
                        )
                    # f32r own half (keys 0:NQ = the core's queries)
                    for nch in range(2):
                        sl = slice(nch * 512, (nch + 1) * 512)
                        xt_c = xtc[nch]
                        for ot in range(KT):
                            ps = ps1.tile([128, 512], F32)
                            for kt in range(KT):
                                nc.tensor.matmul(
                                    ps[:],
                                    wspin_sb[:, kt, ot * 128:(ot + 1) * 128],
                                    xt_c[:, kt, :],
                                    start=(kt == 0), stop=(kt == KT - 1),
                                )
                            # bias add (DVE) + rowsum chunk; ACT makes ST8
                            nc.vector.tensor_scalar(
                                out=ST[:, ot, sl],
                                in0=ps[:],
                                scalar1=bspin_sb[:, ot:ot + 1],
                                scalar2=None,
                                op0=ADD, op1=ADD,
                                accum_out=mf4[:, ot, nch:nch + 1],
                            )
                            nc.scalar.activation(
                                out=ST8[:, ot, sl], in_=ps[:],
                                func=IDENT, bias=bspin_sb[:, ot:ot + 1], scale=1.0,
                            )
                for ot in range(KT):
                    nc.vector.reduce_sum(
                        out=mf[:, ot:ot + 1], in_=mf4[:, ot, :],
                        axis=mybir.AxisListType.X,
                    )
                # mfs8 = 16*mf (e4m3 normal range); g psum then carries
                # WSC*16 = 4096x, copied out with 1/16 -> gT8 = 256*g
                nc.vector.tensor_scalar_mul(mfs8[:], mf[:], 16.0 / N)

            # ---- Phase 2: EF8 = (W_J8^T . ST8)/WSC  (DoubleRow fp8) ----
            EF8 = efp.tile([128, KT, NQ], FP8)  # [d-in-tile, d-tile, query]
            with tc.tile_pool(name="ph2", bufs=1) as ph2:
                wglob8_sb = ph2.tile([128, KT, D], FP8)
                nc.sync.dma_start(out=wglob8_sb[:], in_=wglob_d[:])

                for ot in range(KT):
                    for ch in range(2):
                        ps = ps1.tile([128, 512], F32)
                        for dt_ in range(0, KT, 2):
                            nc.tensor.matmul(
                                ps[:],
                                wj8_sb[:, dt_:dt_ + 2, ot * 128:(ot + 1) * 128],
                                ST8[:, dt_:dt_ + 2, ch * 512:(ch + 1) * 512],
                                start=(dt_ == 0), stop=(dt_ == KT - 2),
                                perf_mode=DR,
                            )
                        nc.vector.tensor_scalar_mul(
                            EF8[:, ot, ch * 512:(ch + 1) * 512], ps[:], 1.0 / WSC
                        )

                # g^T = mf^T . W_global^T (fp8, tiny; rides the rotating tag)
                for ch in range(2):
                    gp = ps1.tile([128, 512], F32, name="ps", tag="ps")
                    for dt_ in range(KT):
                        nc.tensor.matmul(
                            gp[0:1, :],
                            mfs8[:, dt_:dt_ + 1],
                            wglob8_sb[:, dt_, ch * 512:(ch + 1) * 512],
                            start=(dt_ == 0), stop=(dt_ == KT - 1),
                        )
                    nc.vector.tensor_scalar_mul(
                        gT8[:, ch * 512:(ch + 1) * 512], gp[0:1, :], 1.0 / 16.0
                    )
            ps12_cm.__exit__(None, None, None)
            wjp_cm.__exit__(None, None, None)

            # ---- Phase 3+4+5: scores (fp8 DR) -> softmax -> transposes ->
            # pipelined P-transpose + local-field (fp8 DR) ----
            with tc.tile_pool(name="att_sn", bufs=1) as att_sn:
                SN8 = att_sn.tile([128, MT, D], FP8)   # [key-in-tile, key-tile, d]
                SQ = att_sn.tile([128, QT, D], F32)    # 256*s for own queries
                with (
                    tc.tile_pool(name="work", bufs=2) as work,
                    tc.tile_pool(name="ps5s", bufs=1, space="PSUM") as ps5s,
                ):
                    def scores_softmax(qt):
                        q0 = qt * 128
                        ps_s = ps5s.tile([128, NCH, 512], F32)
                        P_sb = work.tile([128, N], FP8, bufs=3)
                        rs4 = stats.tile([128, NCH], F32)
                        for mch in range(NCH):
                            for dt_ in range(0, KT, 2):
                                nc.tensor.matmul(
                                    ps_s[:, mch, :],
                                    EF8[:, dt_:dt_ + 2, q0:q0 + 128],
                                    ST8[:, dt_:dt_ + 2, mch * 512:(mch + 1) * 512],
                                    start=(dt_ == 0), stop=(dt_ == KT - 2),
                                    perf_mode=DR,
                                )
                            # no max subtraction: |scores|*SCALE < ~2 here
                            nc.scalar.activation(
                                out=P_sb[:, mch * 512:(mch + 1) * 512],
                                in_=ps_s[:, mch, :],
                                func=mybir.ActivationFunctionType.Exp,
                                bias=0.0, scale=float(SCALE),
                                accum_out=rs4[:, mch:mch + 1],
                            )
                        rs = stats.tile([128, 1], F32)
                        nc.vector.reduce_sum(out=rs[:], in_=rs4[:],
                                             axis=mybir.AxisListType.X)
                        rinv = stats.tile([128, 1], F32)
                        nc.vector.reciprocal(rinv[:], rs[:])
                        nc.vector.tensor_scalar_mul(rinv[:], rinv[:], PSC)
                        nc.vector.tensor_scalar_mul(P_sb[:], P_sb[:], rinv[:])
                        return P_sb

                    live = {}
                    live[0] = scores_softmax(0)
                    live[1] = scores_softmax(1)

                    # transposes: own-query tiles go via f32r (SQ needs f32
                    # precision); the rest transpose ST8 directly in fp8.
                    with tc.tile_pool(name="ps3", bufs=2, space="PSUM") as ps3:
                        for mt in range(MT):
                            own = mt < QT
                            for dq in range(KT // 4):
                                dsl4 = slice(dq * 512, (dq + 1) * 512)
                                if own:
                                    tp = ps3.tile([128, 4, 128], BF16)
                                    for j in range(4):
                                        nc.tensor.transpose(
                                            tp[:, j, :],
                                            ST[:, dq * 4 + j,
                                               mt * 128:(mt + 1) * 128],
                                            ident_s[:],
                                        )
                                    nc.scalar.copy(SN8[:, mt, dsl4], tp[:])
                                    nc.vector.tensor_scalar_mul(
                                        SQ[:, mt, dsl4], tp[:], PSC)
                                else:
                                    # fp8 PE transpose needs element step 2
                                    # in the PSUM output AP (walrus verifier)
                                    tp8 = ps3.tile([128, 4, 128, 2], FP8)
                                    for j in range(4):
                                        nc.tensor.transpose(
                                            tp8[:, j, :, 0],
                                            ST8[:, dq * 4 + j,
                                                mt * 128:(mt + 1) * 128],
                                            ident8[:],
                                        )
                                    if dq % 2 == 0:
                                        nc.scalar.copy(
                                            SN8[:, mt, dsl4], tp8[:, :, :, 0])
                                    else:
                                        nc.vector.tensor_copy(
                                            SN8[:, mt, dsl4], tp8[:, :, :, 0])

                    ps5t_cm = tc.tile_pool(name="ps5t", bufs=2, space="PSUM")
                    ps5t = ps5t_cm.__enter__()
                    ps5l_cm = tc.tile_pool(name="ps5l", bufs=2, space="PSUM")
                    ps5l = ps5l_cm.__enter__()

                    def pt_lf(qt, P_sb, fine=False):
                        q0 = qt * 128
                        PT = work.tile([128, MT, 128], FP8)
                        for mq in range(MT // 4):
                            tp2 = ps5t.tile([128, 4, 128, 2], FP8)
                            for j in range(4):
                                mt = mq * 4 + j
                                nc.tensor.transpose(
                                    tp2[:, j, :, 0],
                                    P_sb[:, mt * 128:(mt + 1) * 128],
                                    ident8[:],
                                )
                            nc.vector.tensor_copy(
                                PT[:, mq * 4:(mq + 1) * 4, :], tp2[:, :, :, 0])
                        # the very last tile runs at 4x256 so its final
                        # tanh+store chain (the kernel tail) is shorter
                        nd = 4 if fine else 2
                        w = D // nd
                        for dch in range(nd):
                            dsl = slice(dch * w, (dch + 1) * w)
                            plf = ps5l.tile([128, 512], F32)
                            # g term (broadcast over rows via rank-1 matmul)
                            nc.tensor.matmul(
                                plf[:, 0:w], ones8[:], gT8[:, dsl],
                                start=True, stop=False,
                            )
                            for mt in range(0, MT, 2):
                                nc.tensor.matmul(
                                    plf[:, 0:w], PT[:, mt:mt + 2, :],
                                    SN8[:, mt:mt + 2, dsl],
                                    start=False, stop=(mt == MT - 2),
                                    perf_mode=DR,
                                )
                            # psum = 256*(g+LF); add 256*s, tanh(beta/256 * .)
                            z = work.tile([128, 512], F32)
                            nc.vector.tensor_add(
                                z[:, 0:w], plf[:, 0:w], SQ[:, qt, dsl])
                            osb = work.tile([128, 512], F32, name="osb",
                                            tag="osb", bufs=4)
                            nc.scalar.activation(
                                out=osb[:, 0:w], in_=z[:, 0:w],
                                func=mybir.ActivationFunctionType.Tanh,
                                bias=0.0, scale=beta_sb[:],
                            )
                            nc.sync.dma_start(
                                out=out_d[q0:q0 + 128, dsl], in_=osb[:, 0:w])

                    for i in range(2, QT + 2):
                        if i < QT:
                            live[i] = scores_softmax(i)
                        pt_lf(i - 2, live.pop(i - 2), fine=(i == QT + 1))
                    ps5l_cm.__exit__(None, None, None)
                    ps5t_cm.__exit__(None, None, None)

            efp_cm.__exit__(None, None, None)

    nc.compile()
    if hw:
        nc.m = get_hw_module(nc.m)
    return nc


def _tile_kxm(a, np_dt):
    """(K, M) row-major -> [128, K//128, M] with k = kt*128 + p."""
    k, m = a.shape
    return np.ascontiguousarray(
        a.reshape(k // 128, 128, m).transpose(1, 0, 2)
    ).astype(np_dt)


def kernel(x, W_spin, b_spin, W_global, W_J, beta):
    global LAST_RESULT
    x = np.asarray(x, dtype=np.float32)
    W_spin = np.asarray(W_spin, dtype=np.float32)
    b_spin = np.asarray(b_spin, dtype=np.float32)
    W_global = np.asarray(W_global, dtype=np.float32)
    W_J = np.asarray(W_J, dtype=np.float32)
    beta = np.asarray(beta, dtype=np.float32)

    if "fp8" not in _CACHED:
        _CACHED["fp8"] = _build()
    nc = _CACHED["fp8"]

    wspinT = _tile_kxm(W_spin.T, ml_dtypes.bfloat16)   # W_spin.T is (k, o)
    wspinT8 = _tile_kxm(WSC * W_spin.T, ml_dtypes.float8_e4m3)
    wjT8 = _tile_kxm(WSC * W_J.T, ml_dtypes.float8_e4m3)
    wglobT8 = _tile_kxm(WSC * W_global.T, ml_dtypes.float8_e4m3)
    bspin = np.ascontiguousarray(b_spin.reshape(KT, 128).T).astype(np.float32)
    beta_h = np.broadcast_to(beta.reshape(1, 1), (128, 1)).astype(np.float32)
    beta_h = np.ascontiguousarray(beta_h)
    ident_h = np.eye(128, dtype=np.float32)

    # per-half x^T tiles: core (b,h) computes s in f32r for its own half
    # and in fp8 for the peer half
    xt_half_f = [_tile_kxm(np.ascontiguousarray(x[b, h * NQ:(h + 1) * NQ].T),
                           np.float32)
                 for b in range(B) for h in range(2)]
    xt_half = [t.astype(ml_dtypes.bfloat16) for t in xt_half_f]
    xt8_half = [t.astype(ml_dtypes.float8_e4m3) for t in xt_half_f]

    in_maps = []
    for core in range(8):
        b, h = divmod(core, 2)
        in_maps.append({
            "xt": xt_half[2 * b + h], "xt8": xt8_half[2 * b + (1 - h)],
            "wspinT": wspinT, "wspinT8": wspinT8,
            "wjT8": wjT8, "wglobT8": wglobT8,
            "bspin": bspin, "beta": beta_h, "ident": ident_h,
        })

    LAST_RESULT = bass_utils.run_bass_kernel_spmd(
        nc, in_maps, core_ids=list(range(8))
    )

    out = np.empty((B, N, D), dtype=np.float32)
    for core in range(8):
        b, h = divmod(core, 2)
        out[b, h * NQ:(h + 1) * NQ, :] = LAST_RESULT.results[core]["out"]
    return out


# revision 47
# speedup vs baseline: 1.2379x; 1.0174x over previous
"""EmergentSpinGlass fused kernel for 8 Trainium2 NeuronCores.

Reference computation (per batch b):
    s   = x @ W_spin.T + b_spin                       (N, D)
    mf  = mean_n s                                    (D,)
    g   = W_global @ mf                               (D,)   [same for all rows]
    EF  = s @ W_J.T                                   (N, D)
    A   = softmax(EF @ s.T / sqrt(D), axis=-1)        (N, N)
    LF  = A @ s                                       (N, D)
    out = tanh(beta * (s + g + LF))                   (N, D)

Sharding: 8 cores = 4 batches x 2 query-halves. Each core receives x^T for
its batch with its query half's rows permuted first (attention is
permutation-invariant over keys), computes s for all 2048 keys, and runs
the attention block for its 1024 queries. Weights are pre-transposed on
the host; all device matmuls contract over the SBUF partition dim.

Precision plan (v2, fp8 attention):
  - s-path (x, W_spin, s^T) stays float32r (tf32-like): the s term enters
    tanh directly, so it needs ~1e-3 accuracy.
  - EF, scores and the local-field matmuls run in fp8e4m3 with
    perf_mode=DoubleRow (2 contraction k-tiles per instruction, ~1.4x PE
    throughput at 512-wide moving operands). W_J / W_global are pre-scaled
    by 256 on the host so their entries sit in e4m3's normal range; the
    extra 256 is divided out when copying EF/g from PSUM.
  - P (softmax weights ~1/2048) is scaled by 256 before the fp8 cast so it
    stays in e4m3 normal/denormal range; the local-field PSUM is therefore
    256*(g + LF). The s term is added as SQ = 256*s (f32) and tanh uses
    beta/256 as its scale. Predicted end-to-end rel err ~1.2e-3 (numpy
    simulation of the quantization pipeline), vs the 2e-2 gate.

Structure, tuned from hardware profiles:
  - head: the first matmul pass needs only W_spin[kt0:2] + x chunk0[kt0:2]
    (1.5MB); x chunks stream on the ACT HWDGE queue while weights use the
    sync queue, and chunk 0 is loaded in kt-groups so phase 1 starts as
    soon as the first 1.5MB lands (bare-queue baseline waited 20us).
  - ST8 (fp8 copy of s^T for the attention matmuls) is produced by an ACT
    Identity pass reading the same phase-1 PSUM the DVE bias-add reads, so
    it costs no extra DVE time and no serial conversion pass.
  - phase 5 is software-pipelined: scores/exp of query tile i overlap the
    P-transpose + local-field matmuls of tile i-1.
  - PE transposes write 4 tiles into one PSUM bank before a single
    512-wide copy (copy cost is latency-dominated).
"""

import numpy as np
import ml_dtypes

import concourse.bass as bass
import concourse.tile as tile
from concourse import bacc, mybir
from concourse import bass_utils
from concourse.bass_interp import get_hw_module

F32 = mybir.dt.float32
BF16 = mybir.dt.bfloat16
F32R = mybir.dt.float32r
FP8 = mybir.dt.float8e4
ADD = mybir.AluOpType.add
MULT = mybir.AluOpType.mult
DR = mybir.MatmulPerfMode.DoubleRow
IDENT = mybir.ActivationFunctionType.Identity

B, N, D = 4, 2048, 1024
NQ = N // 2          # queries per core
KT = D // 128        # 8 contraction tiles
MT = N // 128        # 16 key tiles
QT = NQ // 128       # 8 query tiles
NCH = N // 512       # 4 key chunks of 512
SCALE = 1.0 / np.sqrt(np.float32(D))
WSC = 256.0          # host pre-scale on W_J / W_global (e4m3 range)
PSC = 256.0          # P scale before fp8 cast

LAST_RESULT = None   # BassKernelResults of the most recent run (for test.py)
_CACHED = {}


def _build(debug=False, hw=True):
    nc = bacc.Bacc(
        "TRN2",
        target_bir_lowering=False,
        debug=False,
        enable_asserts=False,
        num_devices=8,
    )
    xt_d = nc.dram_tensor("xt", [128, KT, NQ], BF16, kind="ExternalInput").ap()
    xt8_d = nc.dram_tensor("xt8", [128, KT, NQ], FP8, kind="ExternalInput").ap()
    wspin_d = nc.dram_tensor("wspinT", [128, KT, D], BF16, kind="ExternalInput").ap()
    wspin8_d = nc.dram_tensor("wspinT8", [128, KT, D], FP8, kind="ExternalInput").ap()
    wj_d = nc.dram_tensor("wjT8", [128, KT, D], FP8, kind="ExternalInput").ap()
    wglob_d = nc.dram_tensor("wglobT8", [128, KT, D], FP8, kind="ExternalInput").ap()
    bspin_d = nc.dram_tensor("bspin", [128, KT], F32, kind="ExternalInput").ap()
    beta_d = nc.dram_tensor("beta", [128, 1], F32, kind="ExternalInput").ap()
    ident_d = nc.dram_tensor("ident", [128, 128], F32, kind="ExternalInput").ap()
    out_d = nc.dram_tensor("out", [NQ, D], F32, kind="ExternalOutput").ap()

    with tile.TileContext(nc) as tc:
        with (
            tc.tile_pool(name="const", bufs=1) as const,
            tc.tile_pool(name="longp", bufs=1) as longp,
            tc.tile_pool(name="stats", bufs=8) as stats,
        ):
            # warmup operand first: DVE memset runs right after the NEFF
            # starts. No gpsimd anywhere in this kernel: gpsimd has ~6us of
            # boot latency that otherwise gates the init barrier, so the
            # identity comes from DRAM and beta arrives pre-broadcast.
            warm8 = const.tile([128, 128], FP8)
            nc.vector.memset(warm8, 1.0)
            ident32 = const.tile([128, 128], F32)
            nc.sync.dma_start(out=ident32[:], in_=ident_d[:])
            ident_s = const.tile([128, 128], BF16)
            nc.vector.tensor_copy(ident_s[:], ident32[:])
            ident8 = const.tile([128, 128], FP8)
            nc.vector.tensor_copy(ident8[:], ident32[:])
            ones8 = const.tile([1, 128], FP8)
            nc.vector.memset(ones8, 1.0)
            beta_sb = const.tile([128, 1], F32)
            nc.sync.dma_start(out=beta_sb[:], in_=beta_d[:])
            # tanh scale: beta/PSC (the local-field psum carries a PSC factor)
            nc.vector.tensor_scalar_mul(beta_sb[:], beta_sb[:], 1.0 / PSC)
            bspin_sb = const.tile([128, KT], F32)
            nc.sync.dma_start(out=bspin_sb[:], in_=bspin_d[:])
            mf4 = const.tile([128, KT, NCH], F32)
            mf = const.tile([128, KT], F32)
            mfs8 = const.tile([128, KT], FP8)
            gT8 = const.tile([1, D], FP8)

            # s^T own key-half (= own queries) in f32r; full s^T in fp8.
            # keys 0:NQ are the core's own rows, NQ:N the peer half — the
            # peer half is only ever consumed in fp8 (scores rhs / SN), so
            # it is computed directly in fp8 from host-quantized x8/W8.
            ST = longp.tile([128, KT, NQ], BF16)  # s^T: [d-in-tile, d-tile, key]
            ST8 = longp.tile([128, KT, N], FP8)   # fp8 s^T, all keys

            # pool stack: efp {EF8} > wjp {wj8} > ph1 {x chunks, wspin}
            efp_cm = tc.tile_pool(name="efp", bufs=1)
            efp = efp_cm.__enter__()
            wjp_cm = tc.tile_pool(name="wjp", bufs=1)
            wjp = wjp_cm.__enter__()

            # ---- Phase 1: s^T = W_spin^T(kxo) . x^T(kxn) + bias; mf; ST8.
            # The fp8 peer half runs FIRST: its critical DMA is 0.375MB
            # (wspin8[0:2] + x8[0:2]) so the PE starts right after launch
            # while the 6MB bf16 stream for the own half lands underneath.
            # One PSUM pool spans all of phase 1 + 2 (no pool-drain barriers
            # between phases): "psA" = 4 accumulators for chunk A's first
            # ot-wave kt-split, "ps" = rotating tiles for everything else.
            ps12_cm = tc.tile_pool(name="ps12", bufs=4, space="PSUM")
            ps1 = ps12_cm.__enter__()

            with tc.tile_pool(name="ph1", bufs=1) as ph1:
                wspin_sb = ph1.tile([128, KT, D], BF16)
                wspin8_sb = ph1.tile([128, KT, D], FP8)
                xt8c = ph1.tile([128, KT, NQ], FP8)
                xtc = {}

                def load_chunk(nch, k0=0, k1=KT):
                    if k0 == 0:
                        t = ph1.tile([128, KT, 512], BF16, name=f"xtc{nch}",
                                     tag="xtc", bufs=2)
                        xtc[nch] = t
                    t = xtc[nch]
                    # x streams on the ACT HWDGE queue; weights on sync
                    nc.scalar.dma_start(
                        out=t[:, k0:k1, :],
                        in_=xt_d[:, k0:k1, nch * 512:(nch + 1) * 512])

                # strict need-order on both queues: fp8 weights/x first
                # (tiny), then the f32r stream interleaved by kt-group.
                nc.sync.dma_start(out=wspin8_sb[:, 0:2, :], in_=wspin8_d[:, 0:2, :])
                nc.scalar.dma_start(out=xt8c[:, 0:2, :], in_=xt8_d[:, 0:2, :])
                nc.sync.dma_start(out=wspin8_sb[:, 2:8, :], in_=wspin8_d[:, 2:8, :])
                nc.scalar.dma_start(out=xt8c[:, 2:8, :], in_=xt8_d[:, 2:8, :])
                nc.sync.dma_start(out=wspin_sb[:, 0:2, :], in_=wspin_d[:, 0:2, :])
                load_chunk(0, 0, 2)
                nc.sync.dma_start(out=wspin_sb[:, 2:4, :], in_=wspin_d[:, 2:4, :])
                load_chunk(0, 2, 4)
                nc.sync.dma_start(out=wspin_sb[:, 4:8, :], in_=wspin_d[:, 4:8, :])
                load_chunk(0, 4, 8)
                load_chunk(1)
                wj8_sb = wjp.tile([128, KT, D], FP8)
                nc.sync.dma_start(out=wj8_sb[:], in_=wj_d[:])

                # fp8 peer-half chunk A (keys NQ:NQ+512): ot-wave 0:4 runs
                # kt-split passes on 4 dedicated accumulators so the PE can
                # start on the first 0.375MB of DMA; a PE warmup on memset
                # data precedes it so the HAM clock gate is released
                # (K=8/8) before the real matmuls arrive.
                psA = [ps1.tile([128, 512], F32, name=f"psA{ot}",
                                tag=f"psA{ot}", bufs=1)
                       for ot in range(4)]
                # enough to flip the HAM clock gate (~3.4us of PE
                # activity) without delaying the first data matmuls
                NWARM = 28
                for i in range(NWARM):
                    nc.tensor.matmul(
                        psA[0][:, 0:128], warm8[:], warm8[:],
                        start=(i == 0), stop=(i == NWARM - 1),
                    )
                for kt0, kt1 in ((0, 2), (2, 4), (4, 8)):
                    for ot in range(4):
                        for kt in range(kt0, kt1, 2):
                            nc.tensor.matmul(
                                psA[ot][:],
                                wspin8_sb[:, kt:kt + 2,
                                          ot * 128:(ot + 1) * 128],
                                xt8c[:, kt:kt + 2, 0:512],
                                start=(kt == 0), stop=(kt == KT - 2),
                                perf_mode=DR,
                            )
                for ot in range(4):
                    # ACT: ST8 = psum/WSC + bias, rowsum into mf4
                    nc.scalar.activation(
                        out=ST8[:, ot, NQ:NQ + 512], in_=psA[ot][:],
                        func=IDENT, bias=bspin_sb[:, ot:ot + 1],
                        scale=1.0 / WSC,
                        accum_out=mf4[:, ot, 2:3],
                    )

                if True:
                    # chunk A ot-wave 4:8, then fp8 peer-half chunk B
                    for ot in range(4, KT):
                        ps = ps1.tile([128, 512], F32)
                        for kt in range(0, KT, 2):
                            nc.tensor.matmul(
                                ps[:],
                                wspin8_sb[:, kt:kt + 2, ot * 128:(ot + 1) * 128],
                                xt8c[:, kt:kt + 2, 0:512],
                                start=(kt == 0), stop=(kt == KT - 2),
                                perf_mode=DR,
                            )
                        nc.scalar.activation(
                            out=ST8[:, ot, NQ:NQ + 512], in_=ps[:],
                            func=IDENT, bias=bspin_sb[:, ot:ot + 1],
                            scale=1.0 / WSC,
                            accum_out=mf4[:, ot, 2:3],
                        )
                    for ot in range(KT):
                        ps = ps1.tile([128, 512], F32)
                        for kt in range(0, KT, 2):
                            nc.tensor.matmul(
                                ps[:],
                                wspin8_sb[:, kt:kt + 2, ot * 128:(ot + 1) * 128],
                                xt8c[:, kt:kt + 2, 512:1024],
                                start=(kt == 0), stop=(kt == KT - 2),
                                perf_mode=DR,
                            )
                        nc.scalar.activation(
                            out=ST8[:, ot, NQ + 512:N], in_=ps[:],
                            func=IDENT, bias=bspin_sb[:, ot:ot + 1],
                            scale=1.0 / WSC,
                            accum_out=mf4[:, ot, 3:4],
                        )
                    # f32r own half (keys 0:NQ = the core's queries)
                    for nch in range(2):
                        sl = slice(nch * 512, (nch + 1) * 512)
                        xt_c = xtc[nch]
                        for ot in range(KT):
                            ps = ps1.tile([128, 512], F32)
                            for kt in range(KT):
                                nc.tensor.matmul(
                                    ps[:],
                                    wspin_sb[:, kt, ot * 128:(ot + 1) * 128],
                                    xt_c[:, kt, :],
                                    start=(kt == 0), stop=(kt == KT - 1),
                                )
                            # bias add (DVE) + rowsum chunk; ACT makes ST8
                            nc.vector.tensor_scalar(
                                out=ST[:, ot, sl],
                                in0=ps[:],
                                scalar1=bspin_sb[:, ot:ot + 1],
                                scalar2=None,
                                op0=ADD, op1=ADD,
                                accum_out=mf4[:, ot, nch:nch + 1],
                            )
                            nc.scalar.activation(
                                out=ST8[:, ot, sl], in_=ps[:],
                                func=IDENT, bias=bspin_sb[:, ot:ot + 1], scale=1.0,
                            )
                for ot in range(KT):
                    nc.vector.reduce_sum(
                        out=mf[:, ot:ot + 1], in_=mf4[:, ot, :],
                        axis=mybir.AxisListType.X,
                    )
                # mfs8 = 16*mf (e4m3 normal range); g psum then carries
                # WSC*16 = 4096x, copied out with 1/16 -> gT8 = 256*g
                nc.vector.tensor_scalar_mul(mfs8[:], mf[:], 16.0 / N)

            # ---- Phase 2: EF8 = (W_J8^T . ST8)/WSC  (DoubleRow fp8) ----
            EF8 = efp.tile([128, KT, NQ], FP8)  # [d-in-tile, d-tile, query]
            with tc.tile_pool(name="ph2", bufs=1) as ph2:
                wglob8_sb = ph2.tile([128, KT, D], FP8)
                nc.sync.dma_start(out=wglob8_sb[:], in_=wglob_d[:])

                for ot in range(KT):
                    for ch in range(2):
                        ps = ps1.tile([128, 512], F32)
                        for dt_ in range(0, KT, 2):
                            nc.tensor.matmul(
                                ps[:],
                                wj8_sb[:, dt_:dt_ + 2, ot * 128:(ot + 1) * 128],
                                ST8[:, dt_:dt_ + 2, ch * 512:(ch + 1) * 512],
                                start=(dt_ == 0), stop=(dt_ == KT - 2),
                                perf_mode=DR,
                            )
                        nc.vector.tensor_scalar_mul(
                            EF8[:, ot, ch * 512:(ch + 1) * 512], ps[:], 1.0 / WSC
                        )

                # g^T = mf^T . W_global^T (fp8, tiny; rides the rotating tag)
                for ch in range(2):
                    gp = ps1.tile([128, 512], F32, name="ps", tag="ps")
                    for dt_ in range(KT):
                        nc.tensor.matmul(
                            gp[0:1, :],
                            mfs8[:, dt_:dt_ + 1],
                            wglob8_sb[:, dt_, ch * 512:(ch + 1) * 512],
                            start=(dt_ == 0), stop=(dt_ == KT - 1),
                        )
                    nc.vector.tensor_scalar_mul(
                        gT8[:, ch * 512:(ch + 1) * 512], gp[0:1, :], 1.0 / 16.0
                    )
            ps12_cm.__exit__(None, None, None)
            wjp_cm.__exit__(None, None, None)

            # ---- Phase 3+4+5: scores (fp8 DR) -> softmax -> transposes ->
            # pipelined P-transpose + local-field (fp8 DR) ----
            with tc.tile_pool(name="att_sn", bufs=1) as att_sn:
                SN8 = att_sn.tile([128, MT, D], FP8)   # [key-in-tile, key-tile, d]
                SQ = att_sn.tile([128, QT, D], F32)    # 256*s for own queries
                with (
                    tc.tile_pool(name="work", bufs=2) as work,
                    tc.tile_pool(name="ps5s", bufs=1, space="PSUM") as ps5s,
                ):
                    def scores_softmax(qt):
                        q0 = qt * 128
                        ps_s = ps5s.tile([128, NCH, 512], F32)
                        P_sb = work.tile([128, N], FP8, bufs=3)
                        rs4 = stats.tile([128, NCH], F32)
                        for mch in range(NCH):
                            for dt_ in range(0, KT, 2):
                                nc.tensor.matmul(
                                    ps_s[:, mch, :],
                                    EF8[:, dt_:dt_ + 2, q0:q0 + 128],
                                    ST8[:, dt_:dt_ + 2, mch * 512:(mch + 1) * 512],
                                    start=(dt_ == 0), stop=(dt_ == KT - 2),
                                    perf_mode=DR,
                                )
                            # no max subtraction: |scores|*SCALE < ~2 here
                            nc.scalar.activation(
                                out=P_sb[:, mch * 512:(mch + 1) * 512],
                                in_=ps_s[:, mch, :],
                                func=mybir.ActivationFunctionType.Exp,
                                bias=0.0, scale=float(SCALE),
                                accum_out=rs4[:, mch:mch + 1],
                            )
                        rs = stats.tile([128, 1], F32)
                        nc.vector.reduce_sum(out=rs[:], in_=rs4[:],
                                             axis=mybir.AxisListType.X)
                        rinv = stats.tile([128, 1], F32)
                        nc.vector.reciprocal(rinv[:], rs[:])
                        nc.vector.tensor_scalar_mul(rinv[:], rinv[:], PSC)
                        nc.vector.tensor_scalar_mul(P_sb[:], P_sb[:], rinv[:])
                        return P_sb

                    live = {}
                    live[0] = scores_softmax(0)
                    live[1] = scores_softmax(1)

                    # transposes: own-query tiles go via f32r (SQ needs f32
                    # precision); the rest transpose ST8 directly in fp8.
                    with tc.tile_pool(name="ps3", bufs=2, space="PSUM") as ps3:
                        for mt in range(MT):
                            own = mt < QT
                            for dq in range(KT // 4):
                                dsl4 = slice(dq * 512, (dq + 1) * 512)
                                if own:
                                    tp = ps3.tile([128, 4, 128], BF16)
                                    for j in range(4):
                                        nc.tensor.transpose(
                                            tp[:, j, :],
                                            ST[:, dq * 4 + j,
                                               mt * 128:(mt + 1) * 128],
                                            ident_s[:],
                                        )
                                    nc.scalar.copy(SN8[:, mt, dsl4], tp[:])
                                    nc.vector.tensor_scalar_mul(
                                        SQ[:, mt, dsl4], tp[:], PSC)
                                else:
                                    # fp8 PE transpose needs element step 2
                                    # in the PSUM output AP (walrus verifier)
                                    tp8 = ps3.tile([128, 4, 128, 2], FP8)
                                    for j in range(4):
                                        nc.tensor.transpose(
                                            tp8[:, j, :, 0],
                                            ST8[:, dq * 4 + j,
                                                mt * 128:(mt + 1) * 128],
                                            ident8[:],
                                        )
                                    if dq % 2 == 0:
                                        nc.scalar.copy(
                                            SN8[:, mt, dsl4], tp8[:, :, :, 0])
                                    else:
                                        nc.vector.tensor_copy(
                                            SN8[:, mt, dsl4], tp8[:, :, :, 0])

                    ps5t_cm = tc.tile_pool(name="ps5t", bufs=2, space="PSUM")
                    ps5t = ps5t_cm.__enter__()
                    ps5l_cm = tc.tile_pool(name="ps5l", bufs=2, space="PSUM")
                    ps5l = ps5l_cm.__enter__()

                    def pt_lf(qt, P_sb, fine=False):
                        q0 = qt * 128
                        PT = work.tile([128, MT, 128], FP8)
                        for mq in range(MT // 4):
                            tp2 = ps5t.tile([128, 4, 128, 2], FP8)
                            for j in range(4):
                                mt = mq * 4 + j
                                nc.tensor.transpose(
                                    tp2[:, j, :, 0],
                                    P_sb[:, mt * 128:(mt + 1) * 128],
                                    ident8[:],
                                )
                            nc.vector.tensor_copy(
                                PT[:, mq * 4:(mq + 1) * 4, :], tp2[:, :, :, 0])
                        # the very last tile runs at 4x256 so its final
                        # tanh+store chain (the kernel tail) is shorter
                        nd = 4 if fine else 2
                        w = D // nd
                        for dch in range(nd):
                            dsl = slice(dch * w, (dch + 1) * w)
                            plf = ps5l.tile([128, 512], F32)
                            # g term (broadcast over rows via rank-1 matmul)
                            nc.tensor.matmul(
                                plf[:, 0:w], ones8[:], gT8[:, dsl],
                                start=True, stop=False,
                            )
                            for mt in range(0, MT, 2):
                                nc.tensor.matmul(
                                    plf[:, 0:w], PT[:, mt:mt + 2, :],
                                    SN8[:, mt:mt + 2, dsl],
                                    start=False, stop=(mt == MT - 2),
                                    perf_mode=DR,
                                )
                            # psum = 256*(g+LF); add 256*s, tanh(beta/256 * .)
                            z = work.tile([128, 512], F32)
                            nc.vector.tensor_add(
                                z[:, 0:w], plf[:, 0:w], SQ[:, qt, dsl])
                            osb = work.tile([128, 512], F32, name="osb",
                                            tag="osb", bufs=4)
                            nc.scalar.activation(
                                out=osb[:, 0:w], in_=z[:, 0:w],
                                func=mybir.ActivationFunctionType.Tanh,
                                bias=0.0, scale=beta_sb[:],
                            )
                            nc.sync.dma_start(
                                out=out_d[q0:q0 + 128, dsl], in_=osb[:, 0:w])

                    for i in range(2, QT + 2):
                        if i < QT:
                            live[i] = scores_softmax(i)
                        pt_lf(i - 2, live.pop(i - 2), fine=(i == QT + 1))
                    ps5l_cm.__exit__(None, None, None)
                    ps5t_cm.__exit__(None, None, None)

            efp_cm.__exit__(None, None, None)

    nc.compile()
    if hw:
        nc.m = get_hw_module(nc.m)
    return nc


def _tile_kxm(a, np_dt):
    """(K, M) row-major -> [128, K//128, M] with k = kt*128 + p."""
    k, m = a.shape
    return np.ascontiguousarray(
        a.reshape(k // 128, 128, m).transpose(1, 0, 2)
    ).astype(np_dt)


def kernel(x, W_spin, b_spin, W_global, W_J, beta):
    global LAST_RESULT
    x = np.asarray(x, dtype=np.float32)
    W_spin = np.asarray(W_spin, dtype=np.float32)
    b_spin = np.asarray(b_spin, dtype=np.float32)
    W_global = np.asarray(W_global, dtype=np.float32)
    W_J = np.asarray(W_J, dtype=np.float32)
    beta = np.asarray(beta, dtype=np.float32)

    if "fp8" not in _CACHED:
        _CACHED["fp8"] = _build()
    nc = _CACHED["fp8"]

    wspinT = _tile_kxm(W_spin.T, ml_dtypes.bfloat16)   # W_spin.T is (k, o)
    wspinT8 = _tile_kxm(WSC * W_spin.T, ml_dtypes.float8_e4m3)
    wjT8 = _tile_kxm(WSC * W_J.T, ml_dtypes.float8_e4m3)
    wglobT8 = _tile_kxm(WSC * W_global.T, ml_dtypes.float8_e4m3)
    bspin = np.ascontiguousarray(b_spin.reshape(KT, 128).T).astype(np.float32)
    beta_h = np.broadcast_to(beta.reshape(1, 1), (128, 1)).astype(np.float32)
    beta_h = np.ascontiguousarray(beta_h)
    ident_h = np.eye(128, dtype=np.float32)

    # per-half x^T tiles: core (b,h) computes s in f32r for its own half
    # and in fp8 for the peer half
    xt_half_f = [_tile_kxm(np.ascontiguousarray(x[b, h * NQ:(h + 1) * NQ].T),
                           np.float32)
                 for b in range(B) for h in range(2)]
    xt_half = [t.astype(ml_dtypes.bfloat16) for t in xt_half_f]
    xt8_half = [t.astype(ml_dtypes.float8_e4m3) for t in xt_half_f]

    in_maps = []
    for core in range(8):
        b, h = divmod(core, 2)
        in_maps.append({
            "xt": xt_half[2 * b + h], "xt8": xt8_half[2 * b + (1 - h)],
            "wspinT": wspinT, "wspinT8": wspinT8,
            "wjT8": wjT8, "wglobT8": wglobT8,
            "bspin": bspin, "beta": beta_h, "ident": ident_h,
        })

    LAST_RESULT = bass_utils.run_bass_kernel_spmd(
        nc, in_maps, core_ids=list(range(8))
    )

    out = np.empty((B, N, D), dtype=np.float32)
    for core in range(8):
        b, h = divmod(core, 2)
        out[b, h * NQ:(h + 1) * NQ, :] = LAST_RESULT.results[core]["out"]
    return out
